# revision 14
# baseline (speedup 1.0000x reference)
"""Trainium2 Bass kernel for y = 2*(einsum('bct,oc->bot', pre, W_pre) + b_pre).

Shapes (hardcoded): pre [16, 512, 4096] f32, W_pre [512, 512] f32, b_pre [512] f32.
Sharding: data-parallel over B across 8 cores (2 batches per core).

Per core: out[b, o, t] = 2*(sum_c W[o,c]*pre[b,c,t] + bias[o]) for 2 batches.
PE matmul computes lhsT.T @ rhs with lhsT = W.T tiles [K=128, M=128] and
rhs = pre tiles [K=128, N=512]; accumulate 4 K-tiles into one PSUM bank,
then ScalarE applies out = 2*psum + 2*bias on eviction PSUM->SBUF.
"""

import os
import sys

for _p in ("/opt/trn_rl_repo", "/root/.axon_site/_ro/trn_rl_repo"):
    if os.path.isdir(_p) and _p not in sys.path:
        sys.path.append(_p)

from contextlib import ExitStack

import numpy as np

import concourse.bass as bass
import concourse.tile as tile
from concourse import bacc, mybir
from concourse.bass_utils import run_bass_kernel_spmd

B, C, T = 16, 512, 4096  # batch, channels (in == out), sequence
NCORES = 8
BPC = B // NCORES  # batches per core
P = 128
KT = C // P  # contraction tiles
MT = C // P  # output-channel tiles
NCHUNK = 512  # matmul moving-operand free dim (max for 4-byte dtypes)
NCH = T // NCHUNK
XCHUNK = 1024  # input DMA column granularity (512 KB per transfer)
XCH = T // XCHUNK

# float32: exact, 4 cycles/row on PE. float32r (tf32): 1 cycle/row at N>=256.
MM_DTYPE = mybir.dt.float32r

LAST_RESULT = None  # BassKernelResults of the most recent run (for test harness)
_cache = {}


def _build(mm_dtype):
    # Bacc (not plain Bass): its finalize() runs move_matmul_waits_to_ldweights +
    # generate_event_semaphores, which walrus needs — an fp32 self-loading
    # matmul's implicit LDWEIGHTS tolerates only one semaphore wait.
    nc = bacc.Bacc("TRN2", target_bir_lowering=False, debug=False, num_devices=NCORES)
    # When running tf32 matmuls, the BIR verifier requires matmul inputs to be
    # produced as float32r; declaring the DRAM side as float32r (with the host
    # pre-rounding the payload to tf32) satisfies it without a device-side pass.
    in_dt = mm_dtype if mm_dtype == mybir.dt.float32r else mybir.dt.float32
    pre = nc.dram_tensor("pre", [BPC, C, T], in_dt, kind="ExternalInput").ap()
    wt = nc.dram_tensor("wt", [C, C], in_dt, kind="ExternalInput").ap()
    b2 = nc.dram_tensor("b2", [P, MT], mybir.dt.float32, kind="ExternalInput").ap()
    out = nc.dram_tensor("out", [BPC, C, T], mybir.dt.float32, kind="ExternalOutput").ap()

    with ExitStack() as ctx:
        tc = ctx.enter_context(tile.TileContext(nc))
        wpool = ctx.enter_context(tc.tile_pool(name="w", bufs=1))
        bpool = ctx.enter_context(tc.tile_pool(name="bias", bufs=1))
        xpool = ctx.enter_context(tc.tile_pool(name="x", bufs=24))
        opool = ctx.enter_context(tc.tile_pool(name="o", bufs=8))
        pspool = ctx.enter_context(tc.tile_pool(name="ps", bufs=8, space="PSUM"))

        # W.T resident in SBUF: 4 tiles [128, 512]; lhsT for (kt, mt) is
        # wtiles[kt][:, mt*128:(mt+1)*128]
        wtiles = []
        for kt in range(KT):
            w = wpool.tile([P, C], in_dt, tag=f"w{kt}")
            nc.sync.dma_start(w[:], wt[kt * P : (kt + 1) * P, :])
            wtiles.append(w)

        btile = bpool.tile([P, MT], mybir.dt.float32)
        nc.sync.dma_start(btile[:], b2[:])

        for b in range(BPC):
            # x tiles arrive in [P, XCHUNK] column chunks so PE can start after
            # ~2 MiB instead of waiting for the whole 8 MiB batch.
            xtiles = [[None] * KT for _ in range(XCH)]
            for xc in range(XCH):
                for kt in range(KT):
                    x = xpool.tile([P, XCHUNK], in_dt, tag="x")
                    nc.sync.dma_start(
                        x[:], pre[b, kt * P : (kt + 1) * P, bass.ts(xc, XCHUNK)]
                    )
                    xtiles[xc][kt] = x
            for nch in range(NCH):
                xc, xoff = divmod(nch * NCHUNK, XCHUNK)
                for mt in range(MT):
                    ps = pspool.tile([P, NCHUNK], mybir.dt.float32, tag="ps")
                    for kt in range(KT):
                        lhsT = wtiles[kt][:, mt * P : (mt + 1) * P]
                        rhs = xtiles[xc][kt][:, xoff : xoff + NCHUNK]
                        if mm_dtype != in_dt:
                            lhsT = lhsT.bitcast(mm_dtype)
                            rhs = rhs.bitcast(mm_dtype)
                        nc.tensor.matmul(
                            ps[:], lhsT, rhs, start=(kt == 0), stop=(kt == KT - 1)
                        )
                    ot = opool.tile([P, NCHUNK], mybir.dt.float32, tag="o")
                    # W is pre-scaled by 2 on the host, so only + 2*bias remains.
                    nc.vector.tensor_scalar_add(ot[:], ps[:], btile[:, mt : mt + 1])
                    nc.sync.dma_start(
                        out[b, mt * P : (mt + 1) * P, bass.ts(nch, NCHUNK)], ot[:]
                    )
    # The axon/PJRT exec path serializes nc as-is; finalize here so Bacc's
    # compile passes (register alloc, event-semaphore wait splitting) run.
    nc.finalize()
    return nc


def _round_tf32(a):
    """Round fp32 array to tf32 (10-bit mantissa), round-to-nearest-even."""
    u = a.view(np.uint32)
    r = u + (0xFFF + ((u >> 13) & 1))
    r &= np.uint32(0xFFFFE000)
    # NaN/Inf payloads must not be touched by the carry into the exponent
    special = (u & np.uint32(0x7F800000)) == np.uint32(0x7F800000)
    r[special] = u[special] & np.uint32(0xFFFFE000)
    return r.view(np.float32)


def kernel(pre, W_pre, b_pre):
    global LAST_RESULT
    pre = np.ascontiguousarray(pre, dtype=np.float32)
    # Fold the reference's final y+y into the weights/bias: out = (2W)x + 2b.
    wT = np.ascontiguousarray(np.asarray(W_pre, dtype=np.float32).T * 2.0)
    if MM_DTYPE == mybir.dt.float32r:
        pre = _round_tf32(pre)
        wT = _round_tf32(wT)
    b2 = np.ascontiguousarray(
        (2.0 * np.asarray(b_pre, dtype=np.float32)).reshape(MT, P).T
    )
    key = str(MM_DTYPE)
    if key not in _cache:
        _cache[key] = _build(MM_DTYPE)
    nc = _cache[key]
    in_maps = [
        {"pre": pre[i * BPC : (i + 1) * BPC], "wt": wT, "b2": b2}
        for i in range(NCORES)
    ]
    res = run_bass_kernel_spmd(nc, in_maps, list(range(NCORES)))
    LAST_RESULT = res
    return np.ascontiguousarray(
        np.concatenate([res.results[i]["out"] for i in range(NCORES)], axis=0),
        dtype=np.float32,
    )


# revision 17
# speedup vs baseline: 1.0850x; 1.0850x over previous
"""Trainium2 Bass kernel for y = 2*(einsum('bct,oc->bot', pre, W_pre) + b_pre).

Shapes (hardcoded): pre [16, 512, 4096] f32, W_pre [512, 512] f32, b_pre [512] f32.
Sharding: data-parallel over B across 8 cores (2 batches per core).

Per core: out[b, o, t] = 2*(sum_c W[o,c]*pre[b,c,t] + bias[o]) for 2 batches.
PE matmul computes lhsT.T @ rhs with lhsT = W.T tiles [K=128, M=128] and
rhs = pre tiles [K=128, N=512]; accumulate 4 K-tiles into one PSUM bank,
then ScalarE applies out = 2*psum + 2*bias on eviction PSUM->SBUF.
"""

import os
import sys

for _p in ("/opt/trn_rl_repo", "/root/.axon_site/_ro/trn_rl_repo"):
    if os.path.isdir(_p) and _p not in sys.path:
        sys.path.append(_p)

from contextlib import ExitStack

import numpy as np

import concourse.bass as bass
import concourse.tile as tile
from concourse import bacc, mybir
from concourse.bass_utils import run_bass_kernel_spmd

B, C, T = 16, 512, 4096  # batch, channels (in == out), sequence
NCORES = 8
BPC = B // NCORES  # batches per core
P = 128
KT = C // P  # contraction tiles
MT = C // P  # output-channel tiles
NCHUNK = 512  # matmul moving-operand free dim (max for 4-byte dtypes)
NCH = T // NCHUNK
XCHUNK = 1024  # input DMA column granularity (512 KB per transfer)
XCH = T // XCHUNK

# float32: exact, 4 cycles/row on PE. float32r (tf32): 1 cycle/row at N>=256.
MM_DTYPE = mybir.dt.float32r

LAST_RESULT = None  # BassKernelResults of the most recent run (for test harness)
_cache = {}


def _build(mm_dtype):
    # Bacc (not plain Bass): its finalize() runs move_matmul_waits_to_ldweights +
    # generate_event_semaphores, which walrus needs — an fp32 self-loading
    # matmul's implicit LDWEIGHTS tolerates only one semaphore wait.
    nc = bacc.Bacc("TRN2", target_bir_lowering=False, debug=False, num_devices=NCORES)
    # When running tf32 matmuls, the BIR verifier requires matmul inputs to be
    # produced as float32r; declaring the DRAM side as float32r (with the host
    # pre-rounding the payload to tf32) satisfies it without a device-side pass.
    in_dt = mm_dtype if mm_dtype == mybir.dt.float32r else mybir.dt.float32
    pre = nc.dram_tensor("pre", [BPC, C, T], in_dt, kind="ExternalInput").ap()
    wt = nc.dram_tensor("wt", [C, C], in_dt, kind="ExternalInput").ap()
    b2 = nc.dram_tensor("b2", [P, MT], mybir.dt.float32, kind="ExternalInput").ap()
    out = nc.dram_tensor("out", [BPC, C, T], mybir.dt.float32, kind="ExternalOutput").ap()

    with ExitStack() as ctx:
        tc = ctx.enter_context(tile.TileContext(nc))
        wpool = ctx.enter_context(tc.tile_pool(name="w", bufs=1))
        bpool = ctx.enter_context(tc.tile_pool(name="bias", bufs=1))
        xpool = ctx.enter_context(tc.tile_pool(name="x", bufs=20))
        opool = ctx.enter_context(tc.tile_pool(name="o", bufs=8))
        pspool = ctx.enter_context(tc.tile_pool(name="ps", bufs=8, space="PSUM"))

        # W.T resident in SBUF: 4 tiles [128, 512]; lhsT for (kt, mt) is
        # wtiles[kt][:, mt*128:(mt+1)*128]
        wtiles = []
        for kt in range(KT):
            w = wpool.tile([P, C], in_dt, tag=f"w{kt}")
            nc.sync.dma_start(w[:], wt[kt * P : (kt + 1) * P, :])
            wtiles.append(w)

        btile = bpool.tile([P, MT], mybir.dt.float32)
        nc.sync.dma_start(btile[:], b2[:])

        for b in range(BPC):
            # x tiles arrive in [P, XCHUNK] column chunks so PE can start after
            # ~2 MiB instead of waiting for the whole 8 MiB batch.
            xtiles = [[None] * KT for _ in range(XCH)]
            for xc in range(XCH):
                for kt in range(KT):
                    x = xpool.tile([P, XCHUNK], in_dt, tag="x")
                    nc.sync.dma_start(
                        x[:], pre[b, kt * P : (kt + 1) * P, bass.ts(xc, XCHUNK)]
                    )
                    xtiles[xc][kt] = x
            # Output staged in [P, OGROUP*NCHUNK] tiles so stores go out as 1 MiB
            # DMAs on the gpsimd queue (splitting issue load off the sync queue).
            OGROUP = 4
            for og in range(NCH // OGROUP):
                otiles = [
                    opool.tile(
                        [P, OGROUP * NCHUNK], mybir.dt.float32,
                        name=f"o_{b}_{og}_{mt}", tag="o",
                    )
                    for mt in range(MT)
                ]
                for j in range(OGROUP):
                    nch = og * OGROUP + j
                    xc, xoff = divmod(nch * NCHUNK, XCHUNK)
                    for mt in range(MT):
                        ps = pspool.tile([P, NCHUNK], mybir.dt.float32, tag="ps")
                        for kt in range(KT):
                            lhsT = wtiles[kt][:, mt * P : (mt + 1) * P]
                            rhs = xtiles[xc][kt][:, xoff : xoff + NCHUNK]
                            if mm_dtype != in_dt:
                                lhsT = lhsT.bitcast(mm_dtype)
                                rhs = rhs.bitcast(mm_dtype)
                            nc.tensor.matmul(
                                ps[:], lhsT, rhs, start=(kt == 0), stop=(kt == KT - 1)
                            )
                        # W is pre-scaled by 2 on the host, so only + 2*bias remains.
                        nc.vector.tensor_scalar_add(
                            otiles[mt][:, bass.ts(j, NCHUNK)],
                            ps[:],
                            btile[:, mt : mt + 1],
                        )
                for mt in range(MT):
                    nc.gpsimd.dma_start(
                        out[
                            b,
                            mt * P : (mt + 1) * P,
                            bass.ds(og * OGROUP * NCHUNK, OGROUP * NCHUNK),
                        ],
                        otiles[mt][:],
                    )
    # The axon/PJRT exec path serializes nc as-is; finalize here so Bacc's
    # compile passes (register alloc, event-semaphore wait splitting) run.
    nc.finalize()
    return nc


def _round_tf32(a):
    """Round fp32 array to tf32 (10-bit mantissa), round-to-nearest-even."""
    u = a.view(np.uint32)
    r = u + (0xFFF + ((u >> 13) & 1))
    r &= np.uint32(0xFFFFE000)
    # NaN/Inf payloads must not be touched by the carry into the exponent
    special = (u & np.uint32(0x7F800000)) == np.uint32(0x7F800000)
    r[special] = u[special] & np.uint32(0xFFFFE000)
    return r.view(np.float32)


def kernel(pre, W_pre, b_pre):
    global LAST_RESULT
    pre = np.ascontiguousarray(pre, dtype=np.float32)
    # Fold the reference's final y+y into the weights/bias: out = (2W)x + 2b.
    wT = np.ascontiguousarray(np.asarray(W_pre, dtype=np.float32).T * 2.0)
    if MM_DTYPE == mybir.dt.float32r:
        pre = _round_tf32(pre)
        wT = _round_tf32(wT)
    b2 = np.ascontiguousarray(
        (2.0 * np.asarray(b_pre, dtype=np.float32)).reshape(MT, P).T
    )
    key = str(MM_DTYPE)
    if key not in _cache:
        _cache[key] = _build(MM_DTYPE)
    nc = _cache[key]
    in_maps = [
        {"pre": pre[i * BPC : (i + 1) * BPC], "wt": wT, "b2": b2}
        for i in range(NCORES)
    ]
    res = run_bass_kernel_spmd(nc, in_maps, list(range(NCORES)))
    LAST_RESULT = res
    return np.ascontiguousarray(
        np.concatenate([res.results[i]["out"] for i in range(NCORES)], axis=0),
        dtype=np.float32,
    )


# revision 22
# speedup vs baseline: 1.1021x; 1.0158x over previous
"""Trainium2 Bass kernel for y = 2*(einsum('bct,oc->bot', pre, W_pre) + b_pre).

Shapes (hardcoded): pre [16, 512, 4096] f32, W_pre [512, 512] f32, b_pre [512] f32.
Sharding: data-parallel over B across 8 cores (2 batches per core).

Per core: out[b, o, t] = 2*(sum_c W[o,c]*pre[b,c,t] + bias[o]) for 2 batches.
PE matmul computes lhsT.T @ rhs with lhsT = W.T tiles [K=128, M=128] and
rhs = pre tiles [K=128, N=512]; accumulate 4 K-tiles into one PSUM bank,
then ScalarE applies out = 2*psum + 2*bias on eviction PSUM->SBUF.
"""

import os
import sys

for _p in ("/opt/trn_rl_repo", "/root/.axon_site/_ro/trn_rl_repo"):
    if os.path.isdir(_p) and _p not in sys.path:
        sys.path.append(_p)

from contextlib import ExitStack

import numpy as np

import concourse.bass as bass
import concourse.tile as tile
from concourse import bacc, mybir
from concourse.bass_utils import run_bass_kernel_spmd

B, C, T = 16, 512, 4096  # batch, channels (in == out), sequence
NCORES = 8
BPC = B // NCORES  # batches per core
P = 128
KT = C // P  # contraction tiles
MT = C // P  # output-channel tiles
NCHUNK = 512  # matmul moving-operand free dim (max for 4-byte dtypes)
NCH = T // NCHUNK
# Input DMA column chunks: small first chunks so the first matmul group's
# data lands early, bigger later ones to amortize DMA issue overhead.
XCS = [512, 512, 1024, 2048]
# Output store groups (in NCHUNK units) per batch: taper the last batch so the
# final DMAs after the last matmul are small.
OGS = {0: [4, 4], 1: [4, 2, 1, 1]}

# float32: exact, 4 cycles/row on PE. float32r (tf32): 1 cycle/row at N>=256.
MM_DTYPE = mybir.dt.float32r

LAST_RESULT = None  # BassKernelResults of the most recent run (for test harness)
_cache = {}


def _build(mm_dtype):
    # Bacc (not plain Bass): its finalize() runs move_matmul_waits_to_ldweights +
    # generate_event_semaphores, which walrus needs — an fp32 self-loading
    # matmul's implicit LDWEIGHTS tolerates only one semaphore wait.
    nc = bacc.Bacc("TRN2", target_bir_lowering=False, debug=False, num_devices=NCORES)
    # When running tf32 matmuls, the BIR verifier requires matmul inputs to be
    # produced as float32r; declaring the DRAM side as float32r (with the host
    # pre-rounding the payload to tf32) satisfies it without a device-side pass.
    in_dt = mm_dtype if mm_dtype == mybir.dt.float32r else mybir.dt.float32
    pre = nc.dram_tensor("pre", [BPC, C, T], in_dt, kind="ExternalInput").ap()
    wt = nc.dram_tensor("wt", [C, C], in_dt, kind="ExternalInput").ap()
    b2 = nc.dram_tensor("b2", [P, MT], mybir.dt.float32, kind="ExternalInput").ap()
    out = nc.dram_tensor("out", [BPC, C, T], mybir.dt.float32, kind="ExternalOutput").ap()

    with ExitStack() as ctx:
        tc = ctx.enter_context(tile.TileContext(nc))
        wpool = ctx.enter_context(tc.tile_pool(name="w", bufs=1))
        bpool = ctx.enter_context(tc.tile_pool(name="bias", bufs=1))
        xpool = ctx.enter_context(tc.tile_pool(name="x", bufs=2))
        opool = ctx.enter_context(tc.tile_pool(name="o", bufs=8))
        pspool = ctx.enter_context(tc.tile_pool(name="ps", bufs=8, space="PSUM"))

        # W.T resident in SBUF as 16 [128, 128] tiles (64 KB DMAs) so the very
        # first matmul group only waits on 4 of them, not the whole 1 MiB.
        wtiles = [[None] * MT for _ in range(KT)]
        for mt in range(MT):
            for kt in range(KT):
                w = wpool.tile([P, P], in_dt, name=f"w_{kt}_{mt}", tag=f"w{kt}{mt}")
                nc.sync.dma_start(
                    w[:], wt[kt * P : (kt + 1) * P, mt * P : (mt + 1) * P]
                )
                wtiles[kt][mt] = w

        btile = bpool.tile([P, MT], mybir.dt.float32)
        nc.sync.dma_start(btile[:], b2[:])

        # nch -> (x tile index, column offset inside that tile)
        xmap = []
        off = 0
        for xi, xcols in enumerate(XCS):
            for o in range(0, xcols, NCHUNK):
                xmap.append((xi, o))
            off += xcols
        assert len(xmap) == NCH

        for b in range(BPC):
            xtiles = [[None] * KT for _ in range(len(XCS))]
            off = 0
            for xi, xcols in enumerate(XCS):
                for kt in range(KT):
                    # Big trailing chunk single-buffered to stay inside SBUF;
                    # its reload for batch 1 overlaps batch 0's tail compute.
                    x = xpool.tile(
                        [P, xcols], in_dt, name=f"x_{b}_{xi}_{kt}",
                        tag=f"x{xi}_{kt}", bufs=(1 if xi == len(XCS) - 1 else 2),
                    )
                    nc.sync.dma_start(
                        x[:], pre[b, kt * P : (kt + 1) * P, bass.ds(off, xcols)]
                    )
                    xtiles[xi][kt] = x
                off += xcols

            nch = 0
            for og, osize in enumerate(OGS[b]):
                ocols = osize * NCHUNK
                otiles = [
                    opool.tile(
                        [P, ocols], mybir.dt.float32,
                        name=f"o_{b}_{og}_{mt}", tag="o",
                    )
                    for mt in range(MT)
                ]
                obase = nch * NCHUNK
                for j in range(osize):
                    xi, xoff = xmap[nch]
                    for mt in range(MT):
                        ps = pspool.tile([P, NCHUNK], mybir.dt.float32, tag="ps")
                        for kt in range(KT):
                            lhsT = wtiles[kt][mt][:]
                            rhs = xtiles[xi][kt][:, xoff : xoff + NCHUNK]
                            if mm_dtype != in_dt:
                                lhsT = lhsT.bitcast(mm_dtype)
                                rhs = rhs.bitcast(mm_dtype)
                            nc.tensor.matmul(
                                ps[:], lhsT, rhs, start=(kt == 0), stop=(kt == KT - 1)
                            )
                        # W is pre-scaled by 2 on the host, so only + 2*bias
                        # remains; alternate DVE/ACT so neither engine binds.
                        dst = otiles[mt][:, bass.ts(j, NCHUNK)]
                        bias_col = btile[:, mt : mt + 1]
                        if mt % 2 == 0:
                            nc.vector.tensor_scalar_add(dst, ps[:], bias_col)
                        else:
                            nc.scalar.activation(
                                dst,
                                ps[:],
                                mybir.ActivationFunctionType.Identity,
                                bias=bias_col,
                            )
                    nch += 1
                for mt in range(MT):
                    nc.gpsimd.dma_start(
                        out[b, mt * P : (mt + 1) * P, bass.ds(obase, ocols)],
                        otiles[mt][:],
                    )
    # The axon/PJRT exec path serializes nc as-is; finalize here so Bacc's
    # compile passes (register alloc, event-semaphore wait splitting) run.
    nc.finalize()
    return nc


def _round_tf32(a):
    """Round fp32 array to tf32 (10-bit mantissa), round-to-nearest-even."""
    u = a.view(np.uint32)
    r = u + (0xFFF + ((u >> 13) & 1))
    r &= np.uint32(0xFFFFE000)
    # NaN/Inf payloads must not be touched by the carry into the exponent
    special = (u & np.uint32(0x7F800000)) == np.uint32(0x7F800000)
    r[special] = u[special] & np.uint32(0xFFFFE000)
    return r.view(np.float32)


def kernel(pre, W_pre, b_pre):
    global LAST_RESULT
    pre = np.ascontiguousarray(pre, dtype=np.float32)
    # Fold the reference's final y+y into the weights/bias: out = (2W)x + 2b.
    wT = np.ascontiguousarray(np.asarray(W_pre, dtype=np.float32).T * 2.0)
    if MM_DTYPE == mybir.dt.float32r:
        pre = _round_tf32(pre)
        wT = _round_tf32(wT)
    b2 = np.ascontiguousarray(
        (2.0 * np.asarray(b_pre, dtype=np.float32)).reshape(MT, P).T
    )
    key = str(MM_DTYPE)
    if key not in _cache:
        _cache[key] = _build(MM_DTYPE)
    nc = _cache[key]
    in_maps = [
        {"pre": pre[i * BPC : (i + 1) * BPC], "wt": wT, "b2": b2}
        for i in range(NCORES)
    ]
    res = run_bass_kernel_spmd(nc, in_maps, list(range(NCORES)))
    LAST_RESULT = res
    return np.ascontiguousarray(
        np.concatenate([res.results[i]["out"] for i in range(NCORES)], axis=0),
        dtype=np.float32,
    )
